# revision 9
# baseline (speedup 1.0000x reference)
"""Trainium2 Bass kernel for nn_LogicLayer (difflogic LogicLayer forward).

Computation (reference):
    w  = softmax(weights, axis=-1)            # [OUT, 16]
    c  = w @ GATE_M                           # [OUT, 4]
    a  = x[:, idx_a]; b = x[:, idx_b]         # [B, OUT] feature gathers
    out = c0 + c1*a + c2*b + c3*(a*b)

Strategy (8 NeuronCores, feature-parallel, fp16 data plane):
  - x is uploaded transposed AND cast to fp16 (xT [IN, B]) and
    replicated; each core computes OUT/8 = 2048 output features over
    the full batch. fp16 halves all HBM traffic vs f32 (48 MiB/core:
    32 gather-read + 16 write -> ~141 us roofline at 358 GB/s); x in
    [0,1) keeps fp16 quantization error ~5e-4, far under the 2e-2
    gate.
  - Per output feature, dma_gather pulls the two needed xT rows (8 KB
    each) from HBM by int16 index — one descriptor per row, so SWDGE
    descriptor generation (~22 ns/desc on the Q7) stays tiny.
  - Gate coefficients c0..c3 are computed on-device from `weights`
    (exp on ScalarE, strided-AP reductions + small tensor ops on VectorE).
  - out = (c0 + c1*a) + b*(c2 + c3*a): the two parenthesized terms are
    per-partition-scalar affine maps of `a` (ScalarE Identity
    activation), combined by two VectorE tensor_tensor passes.
  - Output written as outT [2048, B] (contiguous 16 KB per partition);
    host unshard transposes back.
"""

import numpy as np

BATCH, IN_DIM, OUT_DIM = 4096, 16384, 16384
N_CORES = 8
F_CORE = OUT_DIM // N_CORES  # 2048 output features per core
P = 128


def _build_nc(in_dim, feat_core, batch):
    """Build + compile the per-core Bass program (SPMD, identical cores)."""
    from contextlib import ExitStack

    import concourse.bacc as bacc
    import concourse.mybir as mybir
    import concourse.tile as tile

    F32 = mybir.dt.float32
    F16 = mybir.dt.float16
    I16 = mybir.dt.int16
    TT = feat_core // P  # feature chunks per core (16)
    mult = mybir.AluOpType.mult
    add = mybir.AluOpType.add
    subtract = mybir.AluOpType.subtract
    Ident = mybir.ActivationFunctionType.Identity

    nc = bacc.Bacc(
        "TRN2", target_bir_lowering=False, debug=False, num_swdge_queues=2
    )
    xT = nc.dram_tensor("xT", [in_dim, batch], F16, kind="ExternalInput")
    w = nc.dram_tensor("w", [feat_core, 16], F32, kind="ExternalInput")
    # combined gather indices: per chunk, 128 idx_a then 128 idx_b
    idx = nc.dram_tensor("idx", [P, 2 * feat_core // 16], I16, kind="ExternalInput")
    outT = nc.dram_tensor("outT", [feat_core, batch], F16, kind="ExternalOutput")

    with tile.TileContext(nc) as tc, ExitStack() as ctx:
        const_pool = ctx.enter_context(tc.tile_pool(name="const", bufs=1))
        g_pool = ctx.enter_context(tc.tile_pool(name="g", bufs=3))
        uv_pool = ctx.enter_context(tc.tile_pool(name="uv", bufs=2))

        idx_sb = const_pool.tile([P, 2 * feat_core // 16], I16, tag="idx")
        nc.sync.dma_start(idx_sb[:], idx[:])

        c0 = const_pool.tile([P, TT], F32, tag="c0")
        c1 = const_pool.tile([P, TT], F32, tag="c1")
        c2 = const_pool.tile([P, TT], F32, tag="c2")
        c3 = const_pool.tile([P, TT], F32, tag="c3")

        # ---------- gate coefficients (small setup, freed after) ----------
        with tc.tile_pool(name="setup", bufs=1) as sp:
            w_sb = sp.tile([P, TT, 16], F32, tag="wsb")
            nc.sync.dma_start(w_sb[:], w[:].rearrange("(t p) g -> p t g", p=P))
            E = sp.tile([P, TT, 16], F32, tag="E")
            nc.scalar.activation(E[:], w_sb[:], mybir.ActivationFunctionType.Exp)

            su = sp.tile([P, TT], F32, tag="su")
            nc.vector.reduce_sum(su[:], E[:], axis=mybir.AxisListType.X)
            r = sp.tile([P, TT], F32, tag="r")
            nc.vector.reciprocal(r[:], su[:])

            c0u = sp.tile([P, TT], F32, tag="c0u")
            nc.vector.reduce_sum(c0u[:], E[:, :, 8:16], axis=mybir.AxisListType.X)

            E4 = E[:].rearrange("p t (g2 g1) -> p t g2 g1", g1=4)
            a1 = sp.tile([P, TT], F32, tag="a1")
            nc.vector.reduce_sum(a1[:], E4[:, :, 0:2, 2:4], axis=mybir.AxisListType.XY)
            b1 = sp.tile([P, TT], F32, tag="b1")
            nc.vector.reduce_sum(b1[:], E4[:, :, 2:4, 0:2], axis=mybir.AxisListType.XY)
            c1u = sp.tile([P, TT], F32, tag="c1u")
            nc.vector.tensor_tensor(c1u[:], a1[:], b1[:], op=subtract)

            a2 = sp.tile([P, TT], F32, tag="a2")
            nc.vector.reduce_sum(a2[:], E[:, :, 4:8], axis=mybir.AxisListType.X)
            b2 = sp.tile([P, TT], F32, tag="b2")
            nc.vector.reduce_sum(b2[:], E[:, :, 8:12], axis=mybir.AxisListType.X)
            c2u = sp.tile([P, TT], F32, tag="c2u")
            nc.vector.tensor_tensor(c2u[:], a2[:], b2[:], op=subtract)

            # c3 = (E1+E8) + (E11+E13) - (E2+E4) - (E7+E14) - 2*(E6-E9)
            def eg(g):
                return E[:, :, g : g + 1]

            p1 = sp.tile([P, TT, 1], F32, tag="p1")
            nc.vector.tensor_tensor(p1[:], eg(1), eg(8), op=add)
            p2 = sp.tile([P, TT, 1], F32, tag="p2")
            nc.vector.tensor_tensor(p2[:], eg(11), eg(13), op=add)
            n1 = sp.tile([P, TT, 1], F32, tag="n1")
            nc.vector.tensor_tensor(n1[:], eg(2), eg(4), op=add)
            n2 = sp.tile([P, TT, 1], F32, tag="n2")
            nc.vector.tensor_tensor(n2[:], eg(7), eg(14), op=add)
            d6 = sp.tile([P, TT, 1], F32, tag="d6")
            nc.vector.tensor_tensor(d6[:], eg(6), eg(9), op=subtract)
            pp = sp.tile([P, TT, 1], F32, tag="pp")
            nc.vector.tensor_tensor(pp[:], p1[:], p2[:], op=add)
            nn_ = sp.tile([P, TT, 1], F32, tag="nn")
            nc.vector.tensor_tensor(nn_[:], n1[:], n2[:], op=add)
            c3a = sp.tile([P, TT, 1], F32, tag="c3a")
            nc.vector.tensor_tensor(c3a[:], pp[:], nn_[:], op=subtract)
            c3u = sp.tile([P, TT, 1], F32, tag="c3u")
            nc.vector.scalar_tensor_tensor(
                c3u[:], d6[:], -2.0, c3a[:], op0=mult, op1=add
            )

            nc.vector.tensor_tensor(c0[:], c0u[:], r[:], op=mult)
            nc.vector.tensor_tensor(c1[:], c1u[:], r[:], op=mult)
            nc.vector.tensor_tensor(c2[:], c2u[:], r[:], op=mult)
            nc.vector.tensor_tensor(c3[:], c3u[:, :, 0], r[:], op=mult)

        # ---------- main gather + FMA loop ----------
        # 2 feature-chunks (512 gathered rows) per dma_gather call to
        # amortize the per-call SWDGE fixed cost.
        for gi in range(TT // 2):
            g_t = g_pool.tile([P, 4, batch], F16, tag="g")
            nc.gpsimd.dma_gather(
                g_t[:], xT[:], idx_sb[:, gi * 32 : (gi + 1) * 32], 512, 512, batch,
                queue_num=gi % 2,
            )
            for k in range(2):
                ci = 2 * gi + k
                a_v = g_t[:, 2 * k, :]
                b_v = g_t[:, 2 * k + 1, :]
                cs = slice(ci, ci + 1)
                # u = c0 + c1*a (ScalarE) ; v = c2 + c3*a (VectorE 4x TS)
                u = uv_pool.tile([P, batch], F16, tag="u")
                nc.scalar.activation(
                    u[:], a_v, Ident, bias=c0[:, cs], scale=c1[:, cs]
                )
                v = uv_pool.tile([P, batch], F16, tag="v")
                nc.vector.tensor_scalar(
                    v[:], a_v, c3[:, cs], c2[:, cs], op0=mult, op1=add
                )
                # v = v*b + u  (VectorE, in place) then write out
                nc.vector.tensor_tensor(v[:], v[:], b_v, op=mult)
                nc.vector.tensor_tensor(v[:], v[:], u[:], op=add)
                nc.sync.dma_start(outT[ci * P : (ci + 1) * P, :], v[:])

    nc.compile()
    return nc


def _pack_idx(idx_a, idx_b, feat_lo, feat_hi):
    """Host-side int16 gather-index buffer for one core.

    Per 128-feature chunk: 128 idx_a then 128 idx_b. dma_gather consumes
    index i from partition i%16, column i//16 (replicated across the 8
    groups of 16 partitions).
    """
    cols = []
    for f0 in range(feat_lo, feat_hi, P):
        ids = np.concatenate(
            [idx_a[f0 : f0 + P], idx_b[f0 : f0 + P]]
        ).astype(np.int16)
        blk = ids.reshape(16, 16)  # [col, partition-within-16]
        cols.append(np.tile(blk.T, (P // 16, 1)))  # [128, 16]
    return np.ascontiguousarray(np.concatenate(cols, axis=1))


_NC_CACHE = {}


def _get_nc():
    key = (IN_DIM, F_CORE, BATCH)
    if key not in _NC_CACHE:
        _NC_CACHE[key] = _build_nc(IN_DIM, F_CORE, BATCH)
    return _NC_CACHE[key]


TRACE = False  # set by dev harness to capture an NTFF profile
LAST_RESULT = None


def kernel(x, weights, idx_a, idx_b):
    global LAST_RESULT
    from concourse.bass_utils import run_bass_kernel_spmd

    x = np.asarray(x, dtype=np.float32)
    weights = np.asarray(weights, dtype=np.float32)
    idx_a = np.asarray(idx_a)
    idx_b = np.asarray(idx_b)

    nc = _get_nc()
    xT = np.ascontiguousarray(x.astype(np.float16).T)
    in_maps = []
    for k in range(N_CORES):
        lo, hi = k * F_CORE, (k + 1) * F_CORE
        in_maps.append(
            {
                "xT": xT,
                "w": np.ascontiguousarray(weights[lo:hi]),
                "idx": _pack_idx(idx_a, idx_b, lo, hi),
            }
        )

    res = run_bass_kernel_spmd(nc, in_maps, list(range(N_CORES)), trace=TRACE)
    LAST_RESULT = res
    out = np.empty((BATCH, OUT_DIM), dtype=np.float32)
    for k in range(N_CORES):
        out[:, k * F_CORE : (k + 1) * F_CORE] = res.results[k]["outT"].T.astype(
            np.float32
        )
    return out



# revision 12
# speedup vs baseline: 1.0775x; 1.0775x over previous
"""Trainium2 Bass kernel for nn_LogicLayer (difflogic LogicLayer forward).

Computation (reference):
    w  = softmax(weights, axis=-1)            # [OUT, 16]
    c  = w @ GATE_M                           # [OUT, 4]
    a  = x[:, idx_a]; b = x[:, idx_b]         # [B, OUT] feature gathers
    out = c0 + c1*a + c2*b + c3*(a*b)

Strategy (8 NeuronCores, feature-parallel, fp16 data plane):
  - x is uploaded transposed AND cast to fp16 (xT [IN, B]) and
    replicated; each core computes OUT/8 = 2048 output features over
    the full batch. fp16 halves all HBM traffic vs f32 (48 MiB/core:
    32 gather-read + 16 write -> ~141 us roofline at 358 GB/s); x in
    [0,1) keeps fp16 quantization error ~5e-4, far under the 2e-2
    gate.
  - Per output feature, dma_gather pulls the two needed xT rows (8 KB
    each) from HBM by int16 index — one descriptor per row, so SWDGE
    descriptor generation (~22 ns/desc on the Q7) stays tiny.
  - Gate coefficients c0..c3 are computed on-device from `weights`
    (exp on ScalarE, strided-AP reductions + small tensor ops on VectorE).
  - out = (c0 + c1*a) + b*(c2 + c3*a): the two parenthesized terms are
    per-partition-scalar affine maps of `a` (ScalarE Identity
    activation), combined by two VectorE tensor_tensor passes.
  - Output written as outT [2048, B] (contiguous 16 KB per partition);
    host unshard transposes back.
"""

import numpy as np

BATCH, IN_DIM, OUT_DIM = 4096, 16384, 16384
N_CORES = 8
F_CORE = OUT_DIM // N_CORES  # 2048 output features per core
P = 128


def _build_nc(in_dim, feat_core, batch):
    """Build + compile the per-core Bass program (SPMD, identical cores)."""
    from contextlib import ExitStack

    import concourse.bacc as bacc
    import concourse.mybir as mybir
    import concourse.tile as tile

    F32 = mybir.dt.float32
    F16 = mybir.dt.float16
    I16 = mybir.dt.int16
    TT = feat_core // P  # feature chunks per core (16)
    mult = mybir.AluOpType.mult
    add = mybir.AluOpType.add
    subtract = mybir.AluOpType.subtract
    Ident = mybir.ActivationFunctionType.Identity

    nc = bacc.Bacc(
        "TRN2", target_bir_lowering=False, debug=False, num_swdge_queues=4
    )
    xT = nc.dram_tensor("xT", [in_dim, batch], F16, kind="ExternalInput")
    w = nc.dram_tensor("w", [feat_core, 16], F32, kind="ExternalInput")
    # combined gather indices: per chunk, 128 idx_a then 128 idx_b
    idx = nc.dram_tensor("idx", [P, 2 * feat_core // 16], I16, kind="ExternalInput")
    outT = nc.dram_tensor("outT", [feat_core, batch], F16, kind="ExternalOutput")

    with tile.TileContext(nc) as tc, ExitStack() as ctx:
        const_pool = ctx.enter_context(tc.tile_pool(name="const", bufs=1))
        g_pool = ctx.enter_context(tc.tile_pool(name="g", bufs=4))
        uv_pool = ctx.enter_context(tc.tile_pool(name="uv", bufs=2))

        idx_sb = const_pool.tile([P, 2 * feat_core // 16], I16, tag="idx")
        nc.sync.dma_start(idx_sb[:], idx[:])

        c0 = const_pool.tile([P, TT], F32, tag="c0")
        c1 = const_pool.tile([P, TT], F32, tag="c1")
        c2 = const_pool.tile([P, TT], F32, tag="c2")
        c3 = const_pool.tile([P, TT], F32, tag="c3")

        # ---------- issue all gathers first ----------
        # Gathers depend only on idx_sb; emitting them before the
        # coefficient setup lets the SWDGE/SDMA pipeline start at ~8 us
        # instead of waiting behind the setup ops. 4 queues keep 4
        # gathers in flight (gen of call i waits for call i-4's DMA ring).
        g_tiles = []
        for ci in range(TT):
            g_t = g_pool.tile([P, 2, batch], F16, tag="g")
            nc.gpsimd.dma_gather(
                g_t[:], xT[:], idx_sb[:, ci * 16 : (ci + 1) * 16], 256, 256, batch,
                queue_num=ci % 4,
            )
            g_tiles.append(g_t)

        # ---------- gate coefficients (tiny; lives in const pool) ----------
        if True:
            sp = const_pool
            w_sb = sp.tile([P, TT, 16], F32, tag="wsb")
            nc.sync.dma_start(w_sb[:], w[:].rearrange("(t p) g -> p t g", p=P))
            E = sp.tile([P, TT, 16], F32, tag="E")
            nc.scalar.activation(E[:], w_sb[:], mybir.ActivationFunctionType.Exp)

            su = sp.tile([P, TT], F32, tag="su")
            nc.vector.reduce_sum(su[:], E[:], axis=mybir.AxisListType.X)
            r = sp.tile([P, TT], F32, tag="r")
            nc.vector.reciprocal(r[:], su[:])

            c0u = sp.tile([P, TT], F32, tag="c0u")
            nc.vector.reduce_sum(c0u[:], E[:, :, 8:16], axis=mybir.AxisListType.X)

            E4 = E[:].rearrange("p t (g2 g1) -> p t g2 g1", g1=4)
            a1 = sp.tile([P, TT], F32, tag="a1")
            nc.vector.reduce_sum(a1[:], E4[:, :, 0:2, 2:4], axis=mybir.AxisListType.XY)
            b1 = sp.tile([P, TT], F32, tag="b1")
            nc.vector.reduce_sum(b1[:], E4[:, :, 2:4, 0:2], axis=mybir.AxisListType.XY)
            c1u = sp.tile([P, TT], F32, tag="c1u")
            nc.vector.tensor_tensor(c1u[:], a1[:], b1[:], op=subtract)

            a2 = sp.tile([P, TT], F32, tag="a2")
            nc.vector.reduce_sum(a2[:], E[:, :, 4:8], axis=mybir.AxisListType.X)
            b2 = sp.tile([P, TT], F32, tag="b2")
            nc.vector.reduce_sum(b2[:], E[:, :, 8:12], axis=mybir.AxisListType.X)
            c2u = sp.tile([P, TT], F32, tag="c2u")
            nc.vector.tensor_tensor(c2u[:], a2[:], b2[:], op=subtract)

            # c3 = (E1+E8) + (E11+E13) - (E2+E4) - (E7+E14) - 2*(E6-E9)
            def eg(g):
                return E[:, :, g : g + 1]

            p1 = sp.tile([P, TT, 1], F32, tag="p1")
            nc.vector.tensor_tensor(p1[:], eg(1), eg(8), op=add)
            p2 = sp.tile([P, TT, 1], F32, tag="p2")
            nc.vector.tensor_tensor(p2[:], eg(11), eg(13), op=add)
            n1 = sp.tile([P, TT, 1], F32, tag="n1")
            nc.vector.tensor_tensor(n1[:], eg(2), eg(4), op=add)
            n2 = sp.tile([P, TT, 1], F32, tag="n2")
            nc.vector.tensor_tensor(n2[:], eg(7), eg(14), op=add)
            d6 = sp.tile([P, TT, 1], F32, tag="d6")
            nc.vector.tensor_tensor(d6[:], eg(6), eg(9), op=subtract)
            pp = sp.tile([P, TT, 1], F32, tag="pp")
            nc.vector.tensor_tensor(pp[:], p1[:], p2[:], op=add)
            nn_ = sp.tile([P, TT, 1], F32, tag="nn")
            nc.vector.tensor_tensor(nn_[:], n1[:], n2[:], op=add)
            c3a = sp.tile([P, TT, 1], F32, tag="c3a")
            nc.vector.tensor_tensor(c3a[:], pp[:], nn_[:], op=subtract)
            c3u = sp.tile([P, TT, 1], F32, tag="c3u")
            nc.vector.scalar_tensor_tensor(
                c3u[:], d6[:], -2.0, c3a[:], op0=mult, op1=add
            )

            nc.vector.tensor_tensor(c0[:], c0u[:], r[:], op=mult)
            nc.vector.tensor_tensor(c1[:], c1u[:], r[:], op=mult)
            nc.vector.tensor_tensor(c2[:], c2u[:], r[:], op=mult)
            nc.vector.tensor_tensor(c3[:], c3u[:, :, 0], r[:], op=mult)

        # ---------- main FMA loop ----------
        for ci in range(TT):
            g_t = g_tiles[ci]
            a_v = g_t[:, 0, :]
            b_v = g_t[:, 1, :]
            cs = slice(ci, ci + 1)
            # u = c0 + c1*a (ScalarE) ; v = c2 + c3*a (VectorE 4x TS)
            u = uv_pool.tile([P, batch], F16, tag="u")
            nc.scalar.activation(u[:], a_v, Ident, bias=c0[:, cs], scale=c1[:, cs])
            v = uv_pool.tile([P, batch], F16, tag="v")
            nc.vector.tensor_scalar(
                v[:], a_v, c3[:, cs], c2[:, cs], op0=mult, op1=add
            )
            # v = v*b + u  (VectorE, in place) then write out
            nc.vector.tensor_tensor(v[:], v[:], b_v, op=mult)
            nc.vector.tensor_tensor(v[:], v[:], u[:], op=add)
            nc.sync.dma_start(outT[ci * P : (ci + 1) * P, :], v[:])

    nc.compile()
    return nc


def _pack_idx(idx_a, idx_b, feat_lo, feat_hi):
    """Host-side int16 gather-index buffer for one core.

    Per 128-feature chunk: 128 idx_a then 128 idx_b. dma_gather consumes
    index i from partition i%16, column i//16 (replicated across the 8
    groups of 16 partitions).
    """
    cols = []
    for f0 in range(feat_lo, feat_hi, P):
        ids = np.concatenate(
            [idx_a[f0 : f0 + P], idx_b[f0 : f0 + P]]
        ).astype(np.int16)
        blk = ids.reshape(16, 16)  # [col, partition-within-16]
        cols.append(np.tile(blk.T, (P // 16, 1)))  # [128, 16]
    return np.ascontiguousarray(np.concatenate(cols, axis=1))


_NC_CACHE = {}


def _get_nc():
    key = (IN_DIM, F_CORE, BATCH)
    if key not in _NC_CACHE:
        _NC_CACHE[key] = _build_nc(IN_DIM, F_CORE, BATCH)
    return _NC_CACHE[key]


TRACE = False  # set by dev harness to capture an NTFF profile
LAST_RESULT = None


def kernel(x, weights, idx_a, idx_b):
    global LAST_RESULT
    from concourse.bass_utils import run_bass_kernel_spmd

    x = np.asarray(x, dtype=np.float32)
    weights = np.asarray(weights, dtype=np.float32)
    idx_a = np.asarray(idx_a)
    idx_b = np.asarray(idx_b)

    nc = _get_nc()
    xT = np.ascontiguousarray(x.astype(np.float16).T)
    in_maps = []
    for k in range(N_CORES):
        lo, hi = k * F_CORE, (k + 1) * F_CORE
        in_maps.append(
            {
                "xT": xT,
                "w": np.ascontiguousarray(weights[lo:hi]),
                "idx": _pack_idx(idx_a, idx_b, lo, hi),
            }
        )

    res = run_bass_kernel_spmd(nc, in_maps, list(range(N_CORES)), trace=TRACE)
    LAST_RESULT = res
    out = np.empty((BATCH, OUT_DIM), dtype=np.float32)
    for k in range(N_CORES):
        out[:, k * F_CORE : (k + 1) * F_CORE] = res.results[k]["outT"].T.astype(
            np.float32
        )
    return out

